# revision 1
# baseline (speedup 1.0000x reference)
"""Trainium2 Bass kernel for nn_Aggregation (sparse block-diagonal attention).

Computation (see reference): a single learned query vector attends, per
(sentence, batch), over that sentence's 32 entity slots:
    k/v = entities @ {Wk,Wv}.T + b;  scores = q . k;  attn = softmax_e(scores)
    ctx = sum_e attn * v;            out = ctx @ Wo.T + bo

Algebraic reductions (same as baseline):
 1. The query is one shared vector, so the K projection folds into a tiny
    fused weight computed on host: scores[t, h] = X[t, :] @ M[:, h] + c_h.
 2. The entity-average commutes with the (linear) V projection:
       ctx[(s,b), d] = sum_c Wv[d, c] * Y[h(d), c, (s,b)],
       Y[h, c, (s,b)] = sum_e attn[s,b,h,e] * X[(s,e,b), c].

Tokens are reordered on host to (sent, batch, entity)
so each aligned 32-token run is one complete attention block and each
128-token group is 4 complete blocks.  The Y contraction then needs no
PSUM accumulation across token groups: per (group j, c-chunk) one matmul
with a block-diagonal attention rhs [128 tok, 4*16 (b',h)] produces the
final Y tile -- 4x fewer PE columns than the baseline's masked
accumulation, and the attention-expand constant shrinks 4x.

Per 512-token super-tile (2 sents, 16 blocks), software-pipelined two
deep (scores/softmax run two super-tiles ahead of Y; xn DMAs trail the
xt DMAs by two super-tiles so the drain is xn-arrival-bound):
  scores^T[h, t'] = M^T @ X^T (+ mask via K=1 matmul)        [PE]
  attn = exp(scores + c_h); attn_n = attn / sum_e            [ACT + DVE]
  wb[t', 64] = (attn_n.T via R4S matmul) * blockdiag mask    [PE + DVE]
  Y^T[c_cs, (j,b',h)] = XN_j,cs.T @ wb_j  (start=stop)       [PE]
  copy PSUM Y -> yt SBUF, alternating ACT/DVE engines
Epilogue interleaved with the stream: ctx^T[d, sb] per sb-quarter
(head-pair col-packed PSUM tiles, all matmuls of a bank before its
bias-adds to avoid false-WAR pacing); out[sb, f] = ctx^T.T @ WoT + bo
per sb-half; OUT DMA'd from the ACT-engine queue so it never blocks
the SP input stream.

All inputs are packed into ONE fp16 blob tensor (fp32 consts as raw
bytes, bitcast on device): per-dispatch buffer marshalling dominates
measured per-exec cost under axon, so fewer operands = faster dispatch.
Matmuls in fp16; output DMA'd as fp16 and cast on host.
Self-contained: hardcodes all shapes from the problem spec.
"""

import numpy as np

import concourse.bass as bass
import concourse.tile as tile
from concourse import bacc, mybir, bass_utils

# Problem constants (from spec / setup_inputs)
D = 1024
H = 16
HD = D // H
N_SENTS = 32
N_ENTS = 32
SE = N_SENTS * N_ENTS
B = 64
N_CORES = 8
BC = B // N_CORES            # batch columns per core
TOK = N_SENTS * N_ENTS * BC  # tokens per core = 8192
ST_TOK = 512                 # tokens per super-tile (2 sents x 8 b x 32 e)
N_ST = TOK // ST_TOK         # 16 super-tiles
SB = N_SENTS * BC            # (s, b) rows per core = 256

F32 = mybir.dt.float32
F16 = mybir.dt.float16

# Single-blob input layout (column offsets into XB [128, NB_COLS], fp16).
# Consolidating all inputs into one tensor cuts per-dispatch buffer
# marshalling, which dominates measured per-exec cost under axon.
XT_OFF = 0                       # [128, 8*TOK]
XN_OFF = XT_OFF + 8 * TOK        # [128, (TOK//128)*D]
WVT_OFF = XN_OFF + (TOK // 128) * D   # [128, 8*D]
WOT_OFF = WVT_OFF + 8 * D        # [128, 8*D]
MW_OFF = WOT_OFF + 8 * D         # [128, 8*H]
R4S_OFF = MW_OFF + 8 * H         # [16, 64]
BM64_OFF = R4S_OFF + 64          # [128, 64]
CH_OFF = BM64_OFF + 64           # [16, 2]   fp32 bytes as 2 fp16
BV_OFF = CH_OFF + 2              # [128, 16] fp32 bytes as 16 fp16
BO_OFF = BV_OFF + 16             # [1, D]
MASKV_OFF = BO_OFF + D           # [1, TOK]
ID_OFF = MASKV_OFF + TOK         # [128, 128] identity (PE transpose rhs)
NB_COLS = ID_OFF + 128

_NC_CACHE = {}


def _build(use_mask=True, repeat=1):
    key = ("nc", use_mask, repeat)
    if key in _NC_CACHE:
        return _NC_CACHE[key]
    nc = bacc.Bacc("TRN2", target_bir_lowering=False, debug=False)

    XB = nc.dram_tensor("XB", [128, NB_COLS], F16, kind="ExternalInput").ap()
    XT = XB[:, XT_OFF:XT_OFF + 8 * TOK]
    XN = XB[:, XN_OFF:XN_OFF + (TOK // 128) * D]
    WVT = XB[:, WVT_OFF:WVT_OFF + 8 * D]
    WOT = XB[:, WOT_OFF:WOT_OFF + 8 * D]
    MW = XB[:, MW_OFF:MW_OFF + 8 * H]
    R4S = XB[:H, R4S_OFF:R4S_OFF + 64]
    BM64 = XB[:, BM64_OFF:BM64_OFF + 64]
    CH16 = XB[:H, CH_OFF:CH_OFF + 2]
    BV16 = XB[:, BV_OFF:BV_OFF + 16]
    BO = XB[:1, BO_OFF:BO_OFF + D]
    MASKV = XB[:1, MASKV_OFF:MASKV_OFF + TOK]
    ID128 = XB[:, ID_OFF:ID_OFF + 128]
    OUT = nc.dram_tensor("OUT", [SB, D], F16, kind="ExternalOutput").ap()

    with tile.TileContext(nc) as tc:
        with (
            tc.tile_pool(name="wpool", bufs=1) as wpool,
            tc.tile_pool(name="xtp", bufs=3) as xtp,
            tc.tile_pool(name="xnp", bufs=5) as xnp,
            tc.tile_pool(name="xtsp", bufs=2) as xtsp,
            tc.tile_pool(name="attnpool", bufs=3) as apool,
            tc.tile_pool(name="ctxpool", bufs=1) as cpool,
            tc.tile_pool(name="psS", bufs=2, space="PSUM") as psS,
            tc.tile_pool(name="psQ", bufs=1, space="PSUM") as psQ,
            tc.tile_pool(name="psY", bufs=2, space="PSUM") as psY,
            tc.tile_pool(name="psCtx", bufs=1, space="PSUM") as psCtx,
            tc.tile_pool(name="psT", bufs=1, space="PSUM") as psT,
            tc.tile_pool(name="psF", bufs=1, space="PSUM") as psF,
        ):
            # ---- constants / weights. The big epilogue weights wvt/wot are
            # DMA'd mid-loop so early super-tile loads aren't queued behind
            # them. ----
            wvt = wpool.tile([128, 8 * D], F16)
            wot = wpool.tile([128, 8 * D], F16)
            mw = wpool.tile([128, 8 * H], F16)
            nc.sync.dma_start(mw[:], MW)
            ch16 = wpool.tile([H, 2], F16)
            ch = ch16[:].bitcast(F32)
            bv16 = wpool.tile([128, 16], F16)
            bv = bv16[:].bitcast(F32)
            bo = wpool.tile([1, D], F16)
            maskv = wpool.tile([1, TOK], F16)
            r4s = wpool.tile([H, 64], F16)
            bm64 = wpool.tile([128, 64], F16)
            ones = wpool.tile([1, 128], F16)
            nc.vector.memset(ones[:], 1.0)
            id128 = wpool.tile([128, 128], F16)

            # Y accumulator: [c-in-chunk, cs(8) * (H * SB) + h * SB + sb]
            yt = cpool.tile([128, 8 * H * SB], F16)
            # ctx^T per half: [d-in-chunk, m0(8) * 128 sb-half]
            ctxT = [cpool.tile([128, 8 * 128], F16, tag=f"ctxT{i}",
                               name=f"ctxT{i}")
                    for i in range(2)]

            attn_t = [None] * N_ST
            wb_t = [None] * N_ST
            xn_t = [None] * N_ST

            xt_t = [None] * N_ST

            def load_xt(st):
                # c-chunks 0..3 from DRAM; 4..7 PE-transposed from the xn
                # tile (cuts X DMA 25%).  The last super-tiles load the full
                # X^T so the drain keeps the short xn-bound critical chain.
                w = 8 * ST_TOK if st >= N_ST - 3 else 4 * ST_TOK
                xt = xtp.tile([128, 8 * ST_TOK], F16, tag="xt")
                nc.sync.dma_start(
                    xt[:, :w], XT[:, st * 8 * ST_TOK:st * 8 * ST_TOK + w])
                xt_t[st] = xt

            def load_xn(st):
                xn = xnp.tile([128, 4 * D], F16, tag="xn")
                nc.sync.dma_start(
                    xn[:], XN[:, st * 4 * D:(st + 1) * 4 * D])
                xn_t[st] = xn

            def stage_scores(st):
                t0 = st * ST_TOK
                load_xt(st)
                xt = xt_t[st]
                # transpose-mode super-tiles need xn now; the final full-xt
                # ones let xn trail by one stage to shorten the drain chain
                if st < N_ST - 3:
                    load_xn(st)
                else:
                    load_xn(st - 1)
                xn = xn_t[st]
                if st == 0:
                    # small consts AFTER the first activation tiles so they
                    # don't delay the first scores matmul
                    nc.sync.dma_start(ch16[:], CH16)
                    nc.sync.dma_start(r4s[:], R4S)
                    nc.sync.dma_start(bm64[:], BM64)
                    if use_mask:
                        nc.sync.dma_start(maskv[:], MASKV)
                    nc.sync.dma_start(bv16[:], BV16)
                    nc.sync.dma_start(bo[:], BO)
                    nc.sync.dma_start(id128[:], ID128)
                if 1 <= st <= 2:   # wvt needed from the first ctx quarter
                    q = (st - 1) * 4
                    nc.sync.dma_start(wvt[:, q * D:(q + 4) * D],
                                      WVT[:, q * D:(q + 4) * D])
                if 3 <= st <= 6:   # wot needed from the first out half
                    q = (st - 3) * 2
                    nc.sync.dma_start(wot[:, q * D:(q + 2) * D],
                                      WOT[:, q * D:(q + 2) * D])

                # ---- on-chip X^T for c-chunks 4..7: PE-transpose the
                # xn tile through a PSUM staging bank (2 chunks per pass) --
                xts = None
                for g in range(2 if st < N_ST - 3 else 0):
                    if xts is None:
                        xts = xtsp.tile([128, 4 * ST_TOK], F16, tag="xts")
                    pst = psT.tile([128, 2 * ST_TOK], F16, tag="pst")
                    for k in range(2):
                        cs = 4 + g * 2 + k
                        for tc_ in range(4):
                            nc.tensor.transpose(
                                pst[:, k * ST_TOK + tc_ * 128:
                                    k * ST_TOK + (tc_ + 1) * 128],
                                xn[:, tc_ * D + cs * 128:
                                   tc_ * D + (cs + 1) * 128],
                                id128[:],
                            )
                    if g == 0:
                        nc.vector.tensor_copy(
                            xts[:, :2 * ST_TOK], pst[:])
                    else:
                        nc.scalar.copy(
                            xts[:, 2 * ST_TOK:], pst[:])

                # ---- scores^T [16 h, 512 t'] = M^T X^T (+ mask) ----
                ps_s = psS.tile([H, ST_TOK], F32, tag="ps_s")
                for c in range(8):
                    rhs = (xt[:, c * ST_TOK:(c + 1) * ST_TOK]
                           if (c < 4 or st >= N_ST - 3) else
                           xts[:, (c - 4) * ST_TOK:(c - 3) * ST_TOK])
                    nc.tensor.matmul(
                        ps_s[:],
                        mw[:, c * H:(c + 1) * H],
                        rhs,
                        start=(c == 0), stop=(c == 7 and not use_mask),
                    )
                if use_mask:
                    nc.tensor.matmul(
                        ps_s[:], ones[:, :H],
                        maskv[:, t0:t0 + ST_TOK],
                        start=False, stop=True,
                    )

                # ---- softmax over e (contiguous runs of 32) ----
                attn = apool.tile([H, ST_TOK], F16, tag="attn")
                nc.scalar.activation(attn[:], ps_s[:],
                                     mybir.ActivationFunctionType.Exp,
                                     bias=ch)
                # softmax tail lives on the (otherwise idle) GpSimd queue:
                # three adjacent ops, so the congested DVE queue never sits
                # on the scores critical path
                zsum = apool.tile([H, 16], F32, tag="zsum")
                nc.vector.reduce_sum(
                    zsum[:],
                    attn[:].rearrange("p (g e) -> p g e", e=N_ENTS),
                    axis=mybir.AxisListType.X)
                zrec = apool.tile([H, 16], F32, tag="zrec")
                nc.vector.reciprocal(zrec[:], zsum[:])
                attn_n = apool.tile([H, ST_TOK], F16, tag="attn_n")
                nc.vector.tensor_mul(
                    attn_n[:].rearrange("p (g e) -> p g e", e=N_ENTS),
                    attn[:].rearrange("p (g e) -> p g e", e=N_ENTS),
                    zrec[:].rearrange("p g -> p g", g=16)[:, :, None]
                    .broadcast_to((H, 16, N_ENTS)),
                )
                attn_t[st] = attn_n

            def stage_wb(st):
                # ---- wb[j]: [128 t', 64 (b', h)] = attn_n.T * blockdiag ----
                # Issued one stage after stage_scores(st) so the in-order PE
                # queue never stalls waiting for st's softmax chain.
                attn_n = attn_t[st]
                wb = apool.tile([128, 4 * 64], F16, tag="wb")
                ps_q = psQ.tile([128, 256], F32, tag="ps_q")
                for j in range(4):
                    nc.tensor.matmul(
                        ps_q[:, j * 64:(j + 1) * 64],
                        attn_n[:, j * 128:(j + 1) * 128], r4s[:],
                        start=True, stop=True,
                    )
                    nc.vector.tensor_mul(
                        wb[:, j * 64:(j + 1) * 64],
                        ps_q[:, j * 64:(j + 1) * 64], bm64[:])
                wb_t[st] = wb
                attn_t[st] = None

            def stage_y(st):
                xn = xn_t[st]
                wb = wb_t[st]
                # ---- Y tiles: one PSUM bank holds a (cs, cs+1) pair ----
                ytv = yt[:].rearrange("p (ch h sb) -> p ch h sb", ch=8, h=H)
                for cp in range(4):
                    ps_y = psY.tile([128, 512], F32, tag="ps_y")
                    for k in range(2):
                        cs = 2 * cp + k
                        for j in range(4):
                            nc.tensor.matmul(
                                ps_y[:, k * 256 + j * 64:
                                     k * 256 + (j + 1) * 64],
                                xn[:, j * D + cs * 128:
                                   j * D + (cs + 1) * 128],
                                wb[:, j * 64:(j + 1) * 64],
                                start=True, stop=True,
                            )
                    # copy into yt: psum col k*256 + j*64 + b'*16 + h
                    #  -> yt col (2cp+k)*(H*SB) + h*SB + st*16 + j*4 + b'
                    src = ps_y[:].rearrange("p (k j bq h) -> p k h (j bq)",
                                            k=2, j=4, bq=4)
                    dst = ytv[:, 2 * cp: 2 * cp + 2, :,
                              st * 16: st * 16 + 16]
                    if cp % 2 == 0:
                        nc.scalar.copy(dst, src)
                    else:
                        nc.vector.tensor_copy(dst, src)
                xn_t[st] = None
                wb_t[st] = None

            def ctx_piece(piece, sb0, w, half, coff):
                # ---- ctx^T: [128 d (2 heads col-packed), w sb] ----
                ctxT_bf = ctxT[half]
                # all matmuls of a 4-m0 bank first, then its adds: a region's
                # reader would otherwise false-WAR the next region's matmuls
                for mg in range(2):
                    ps_ctx = psCtx.tile([128, 4 * w], F32, tag="ps_ctx")
                    for ml in range(4):
                        m0 = mg * 4 + ml
                        reg = ps_ctx[:, ml * w:(ml + 1) * w]
                        for hh in range(2):
                            h = 2 * m0 + hh
                            for c in range(8):
                                nc.tensor.matmul(
                                    reg[hh * 64:(hh + 1) * 64, :],
                                    wvt[:, c * D + h * HD: c * D + h * HD + HD],
                                    yt[:, c * (H * SB) + h * SB + sb0:
                                       c * (H * SB) + h * SB + sb0 + w],
                                    start=(c == 0), stop=(c == 7),
                                    tile_position=(0, hh * 64),
                                )
                    for ml in range(4):
                        m0 = mg * 4 + ml
                        reg = ps_ctx[:, ml * w:(ml + 1) * w]
                        dst = ctxT_bf[:, m0 * 128 + coff: m0 * 128 + coff + w]
                        nc.vector.tensor_add(
                            dst, reg,
                            bv[:, m0:m0 + 1].broadcast_to((128, w)))

            def out_half(half):
                # ---- out projection: OUT[sb, f] = ctx^T.T @ WoT + bo ----
                sb0 = half * 128
                ctxT_bf = ctxT[half]
                fin = cpool.tile([128, D], F16, tag=f"fin{half}",
                                 name=f"fin{half}")
                for nh in range(2):
                    ps_f = psF.tile([128, 512], F32, tag="ps_f")
                    for c in range(8):
                        nc.tensor.matmul(
                            ps_f[:],
                            ctxT_bf[:, c * 128:(c + 1) * 128],
                            wot[:, c * D + nh * 512: c * D + (nh + 1) * 512],
                            start=(c == 0), stop=False,
                        )
                    nc.tensor.matmul(
                        ps_f[:], ones[:, :128],
                        bo[:, nh * 512:(nh + 1) * 512],
                        start=False, stop=True,
                    )
                    nc.scalar.copy(fin[:, nh * 512:(nh + 1) * 512], ps_f[:])
                    # OUT goes out on the ACT-engine DGE queue (per nh
                    # half, so the first DMA overlaps the second half's
                    # matmuls) and never blocks the SP input stream.
                    nc.scalar.dma_start(
                        OUT[sb0:sb0 + 128, nh * 512:(nh + 1) * 512],
                        fin[:, nh * 512:(nh + 1) * 512])

            # ---- main loop, software-pipelined two super-tiles deep.
            # repeat>1 unrolls the whole pipeline (timing builds): the
            # marginal between repeat counts is the steady-state HW time.
            for rep in range(repeat):
              for stage in range(N_ST + 2):
                  if stage < N_ST:
                      stage_scores(stage)
                  if stage == N_ST:
                      load_xn(N_ST - 1)
                  if 1 <= stage <= N_ST:
                      stage_wb(stage - 1)
                  st_y = stage - 2
                  if st_y >= 0:
                      stage_y(st_y)
                      if st_y == 3:
                          ctx_piece(0, 0, 64, 0, 0)
                      elif st_y == 7:
                          ctx_piece(1, 64, 64, 0, 64)
                          out_half(0)
                      elif st_y == 12:
                          ctx_piece(2, 128, 80, 1, 0)
                      elif st_y == 14:
                          ctx_piece(3, 208, 32, 1, 80)
                      elif st_y == 15:
                          ctx_piece(4, 240, 16, 1, 112)
                          out_half(1)

    nc.compile()
    _NC_CACHE[key] = nc
    return nc


def _prep_host(entities, padding_mask, n_sents, query, in_proj_w, in_proj_b,
               out_proj_w, out_proj_b):
    """Host-side prep: shard + layout/dtype packing + weight fusion."""
    assert int(n_sents) == N_SENTS
    f16 = np.float16
    f32 = np.float32

    Wq = in_proj_w[:D]
    Wk = in_proj_w[D:2 * D]
    Wv = in_proj_w[2 * D:]
    bq = in_proj_b[:D]
    bk = in_proj_b[D:2 * D]
    bv = in_proj_b[2 * D:]
    scale = np.float64(1.0) / np.sqrt(np.float64(HD))

    q_vec = ((query.astype(np.float64) @ Wq.T.astype(np.float64)
              + bq.astype(np.float64)) * scale)
    # M[c, h] = sum_hd q_vec[h*HD+hd] * Wk[h*HD+hd, c];  c_h = q_vec_h . bk_h
    M = np.stack(
        [q_vec[h * HD:(h + 1) * HD] @ Wk.astype(np.float64)[h * HD:(h + 1) * HD, :]
         for h in range(H)], axis=1)  # [D, H]
    c_h = np.array(
        [q_vec[h * HD:(h + 1) * HD] @ bk.astype(np.float64)[h * HD:(h + 1) * HD]
         for h in range(H)])

    def pack_kxn(w_t):  # [1024, N] -> [128, 8*N] chunk-major
        n = w_t.shape[1]
        return np.ascontiguousarray(
            w_t.reshape(8, 128, n).transpose(1, 0, 2).reshape(128, 8 * n))

    WVT = pack_kxn(Wv.T.astype(f32)).astype(f16)
    WOT = pack_kxn(out_proj_w.T.astype(f32)).astype(f16)
    MW = pack_kxn(M.astype(f32)).astype(f16)
    CH = c_h.astype(f32).reshape(H, 1)
    BVp = np.ascontiguousarray(bv.astype(f32).reshape(8, 128).T)  # [128, 8]
    BOp = out_proj_b.astype(f32).reshape(1, D).astype(f16)

    # R4S[h, 16*b' + h2] = [h == h2]
    R4Sp = np.zeros((H, 64), dtype=f16)
    for bq_ in range(4):
        for h in range(H):
            R4Sp[h, bq_ * 16 + h] = 1.0
    # BM64[32*b1 + e, 16*b2 + h] = [b1 == b2]
    BM64p = np.zeros((128, 64), dtype=f16)
    for b1 in range(4):
        BM64p[b1 * 32:(b1 + 1) * 32, b1 * 16:(b1 + 1) * 16] = 1.0

    ent16 = entities.astype(f16)  # [SE, B, D]
    maskf = padding_mask.astype(f32) * f32(-30000.0)

    in_maps = []
    for core in range(N_CORES):
        bsl = slice(core * BC, (core + 1) * BC)
        # token order (s, b, e): t = (s*BC + b)*NE + e
        xflat = np.ascontiguousarray(
            ent16[:, bsl, :].reshape(N_SENTS, N_ENTS, BC, D)
            .transpose(0, 2, 1, 3)).reshape(TOK, D)
        # X natural, super-tile-major: [p, st * 4096 + j * D + c]
        xn = np.ascontiguousarray(
            xflat.reshape(N_ST, 4, 128, D).transpose(2, 0, 1, 3)
            .reshape(128, N_ST * 4 * D))
        # X^T, super-tile-major: [p=c-in-chunk, st * 4096 + c_chunk * 512 + t]
        xt = xflat.T.reshape(8, 128, N_ST, ST_TOK)
        xt = np.ascontiguousarray(
            xt.transpose(1, 2, 0, 3).reshape(128, 8 * TOK))
        maskv = np.ascontiguousarray(
            maskf[:, bsl].reshape(N_SENTS, N_ENTS, BC).transpose(0, 2, 1)
            .reshape(1, TOK)).astype(f16)
        xb = np.zeros((128, NB_COLS), dtype=f16)
        xb[:, XT_OFF:XT_OFF + 8 * TOK] = xt
        xb[:, XN_OFF:XN_OFF + (TOK // 128) * D] = xn
        xb[:, WVT_OFF:WVT_OFF + 8 * D] = WVT
        xb[:, WOT_OFF:WOT_OFF + 8 * D] = WOT
        xb[:, MW_OFF:MW_OFF + 8 * H] = MW
        xb[:H, R4S_OFF:R4S_OFF + 64] = R4Sp
        xb[:, BM64_OFF:BM64_OFF + 64] = BM64p
        xb[:H, CH_OFF:CH_OFF + 2] = CH.view(f16)
        xb[:, BV_OFF:BV_OFF + 16] = BVp.view(f16)
        xb[:1, BO_OFF:BO_OFF + D] = BOp
        xb[:1, MASKV_OFF:MASKV_OFF + TOK] = maskv
        xb[:, ID_OFF:ID_OFF + 128] = np.eye(128, dtype=f16)
        in_maps.append({"XB": xb})
    return in_maps


def kernel(entities, padding_mask, n_sents, query, in_proj_w, in_proj_b,
           out_proj_w, out_proj_b):
    # Accept jax/np arrays alike; host prep must run in numpy (and the
    # q/Wk fold in float64, which jax with x64 disabled would silently
    # downcast).
    entities = np.asarray(entities)
    padding_mask = np.asarray(padding_mask)
    query = np.asarray(query)
    in_proj_w = np.asarray(in_proj_w)
    in_proj_b = np.asarray(in_proj_b)
    out_proj_w = np.asarray(out_proj_w)
    out_proj_b = np.asarray(out_proj_b)
    n_sents = int(n_sents)
    in_maps = _prep_host(entities, padding_mask, n_sents, query, in_proj_w,
                         in_proj_b, out_proj_w, out_proj_b)
    nc = _build(use_mask=bool(np.any(padding_mask)))
    res = None
    last_err = None
    for attempt in range(3):
        try:
            res = bass_utils.run_bass_kernel_spmd(
                nc, in_maps=in_maps, core_ids=list(range(N_CORES)))
            break
        except Exception as e:  # rare transient device wedge; retry
            last_err = e
            import time as _time
            _time.sleep(3)
    if res is None:
        raise last_err
    out = np.empty((N_SENTS, B, D), dtype=np.float32)
    for core in range(N_CORES):
        o = res.results[core]["OUT"].astype(np.float32).reshape(
            N_SENTS, BC, D)
        out[:, core * BC:(core + 1) * BC, :] = o
    return out

